# revision 24
# baseline (speedup 1.0000x reference)
"""Multi-head attention (B=4, S=2048, D=1024, H=16, Dh=64) on 8 TRN2 NeuronCores.

Sharding: core c handles batch b = c // 2 and head group g = c % 2 (8 heads
each).  Every core computes Q/K/V projections for its batch+heads, the
attention for those heads, and a *partial* output projection (its heads'
slice of Wo).  The host sums the two partials per batch while unsharding.

v2 dataflow, engineered around three facts measured on hardware:
  - PE cost is (output free-size x cycle) regardless of M/K, and the PE
    only reaches 2.4 GHz after ~3us of *continuous* execution; any stall
    drops it to 1.2 GHz.  So the PE queue must never go idle.
  - Scalar ACT exp of all logits is a ~342us floor; part of the exp work
    is moved to the DVE as a Schraudolph bit-trick exp (affine -> int16
    convert -> bitcast bf16), calibrated to ~1.8% rms which cancels in
    softmax normalization (same approximated weights in numerator via the
    V ones-column denominator).
  - PSUM is 8 banks: logits tiles [128, 2 heads, 512f] double-buffered
    (4 banks) + two ctx accumulators [65, 512] (2) + weave psum (2).

Loop structure: f-chunk (cc) outer, head-pair (j) inner.  Background
matmul work (K/Q projections of later blocks, output projection of the
previous f-chunk) is "woven" between attention matmuls to fill the
PE bubbles left by exp latency, keeping the PE p-state at max.
"""

import sys

sys.path.insert(0, "/opt/trn_rl_repo")

import numpy as np
import ml_dtypes

BF = ml_dtypes.bfloat16

# Problem geometry (hardcoded; the harness always calls with these shapes).
B, S, D, H, Dh = 4, 2048, 1024, 16, 64
N_CORES = 8
H_LOC = H // 2          # heads per core
HK = H_LOC * Dh         # 512

P = 128
J = H_LOC // 2          # head pairs
DC = D // P             # contraction chunks for projections
TT = S // P             # t (key) tiles
W = 512                 # f-chunk width per head
NCC = S // W            # f-chunks
ND = D // 512

SCALE = float(Dh) ** -0.5      # 0.125
SHIFT = -2.0                   # exp(x*SCALE + SHIFT): keeps e in bf16 sweet spot
A16 = 128.0 / np.log(2.0)      # Schraudolph bf16 exponent stuffing
C16 = 7.3                      # sawtooth centering (calibrated)
SCHR_A = SCALE * A16
SCHR_B = 127 * 128 - C16 + SHIFT * A16
# m-tiles whose exp runs on the DVE (Schraudolph); rest on Scalar (exact)
DVE_M = (1, 4, 7, 10, 13)


def build_nc():
    import concourse.mybir as mybir
    import concourse.tile as tile
    from concourse import bacc
    from concourse.bass import ds, ts
    from contextlib import ExitStack

    FP32 = mybir.dt.float32
    BF16 = mybir.dt.bfloat16
    I16 = mybir.dt.int16
    EXP = mybir.ActivationFunctionType.Exp
    COPY = mybir.ActivationFunctionType.Copy
    MULT = mybir.AluOpType.mult
    ADD = mybir.AluOpType.add

    nc = bacc.Bacc("TRN2")
    xq = nc.declare_dram_parameter("xq_t", [D, S], BF16, isOutput=False)
    xk = nc.declare_dram_parameter("xk_t", [D, S], BF16, isOutput=False)
    xv = nc.declare_dram_parameter("xv_t", [D, S], BF16, isOutput=False)
    wq = nc.declare_dram_parameter("wq", [D, HK], BF16, isOutput=False)
    wk = nc.declare_dram_parameter("wk", [D, HK], BF16, isOutput=False)
    wv = nc.declare_dram_parameter("wv", [D, HK], BF16, isOutput=False)
    wo = nc.declare_dram_parameter("wo", [HK, D], BF16, isOutput=False)
    out = nc.declare_dram_parameter("out_part", [S, D], FP32, isOutput=True)

    with tile.TileContext(nc) as tc, ExitStack() as ctx:
        singles = ctx.enter_context(tc.tile_pool(name="singles", bufs=1))

        # ---- persistent SBUF tensors -------------------------------------
        wq_sb = singles.tile([P, DC, HK], BF16, tag="wq", name="wq")
        wk_sb = singles.tile([P, DC, HK], BF16, tag="wk", name="wk")
        wv_sb = singles.tile([P, DC, HK], BF16, tag="wv", name="wv")
        wo_sb = singles.tile([P, J, D], BF16, tag="wo", name="wo")
        xq_sb = singles.tile([P, DC, S], BF16, tag="xq", name="xq")
        xk_sb = singles.tile([P, DC, S], BF16, tag="xk", name="xk")
        qT = [singles.tile([P, S], BF16, tag=f"qT{j}", name=f"qT{j}")
              for j in range(J)]
        kT = [singles.tile([P, S], BF16, tag=f"kT{j}", name=f"kT{j}")
              for j in range(J)]
        vt = [singles.tile([P, H_LOC, Dh + 1], BF16, tag=f"vt{m}", name=f"vt{m}")
              for m in range(TT)]
        bias_sh = singles.tile([P, 1], FP32, tag="bias_sh", name="bias_sh")
        nc.gpsimd.memset(bias_sh, SHIFT)

        # ---- DMA prelude (xv via transient pool, freed after V proj) -----
        def load_w(dst, src, rg="(a p) n -> p a n"):
            nc.sync.dma_start(out=dst, in_=src[:, :].rearrange(rg, p=P))

        def load_x(dst, src):
            src_r = src[:, :].rearrange("(a p) s -> p a s", p=P)
            for dc in range(DC):
                nc.sync.dma_start(out=dst[:, dc, :], in_=src_r[:, dc, :])

        # ---- pools -------------------------------------------------------
        ppl = ctx.enter_context(tc.tile_pool(name="ppl", bufs=2, space="PSUM"))
        ppc = ctx.enter_context(tc.tile_pool(name="ppc", bufs=1, space="PSUM"))
        ppw = ctx.enter_context(tc.tile_pool(name="ppw", bufs=2, space="PSUM"))
        epool = ctx.enter_context(tc.tile_pool(name="epool", bufs=2))
        ctpool = ctx.enter_context(tc.tile_pool(name="ctp", bufs=2))
        stpool = ctx.enter_context(tc.tile_pool(name="stage", bufs=1))
        rpool = ctx.enter_context(tc.tile_pool(name="rbc", bufs=1))
        obpool = ctx.enter_context(tc.tile_pool(name="outb", bufs=1))

        # ---------- weave machinery --------------------------------------
        # Background PE work (projection / output-projection matmuls) fed
        # one instruction at a time into attention's exp-latency bubbles.
        weave_q = []

        def weave(n):
            for _ in range(min(n, len(weave_q))):
                weave_q.pop(0)()

        def drain_all():
            while weave_q:
                weave_q.pop(0)()

        def proj_chain(x_sb, w_sb, jj, cc, dst):
            # kT/qT[jj][:, cc*W:(cc+1)*W] = (W[:, jj] ).T @ xT chunk
            units = []
            pw = [None]

            def mk(dc):
                def emit():
                    if dc == 0:
                        pw[0] = ppw.tile([P, W], FP32, tag="pw", name="pw")
                    nc.tensor.matmul(
                        pw[0], lhsT=w_sb[:, dc, ts(jj, P)],
                        rhs=x_sb[:, dc, ds(cc * W, W)],
                        start=(dc == 0), stop=(dc == DC - 1))
                return emit
            for dc in range(DC):
                units.append(mk(dc))

            def drain():
                # DVE (gpsimd cannot read PSUM); Scalar stays exp-only
                nc.vector.tensor_copy(out=dst[:, ds(cc * W, W)], in_=pw[0])
            units.append(drain)
            return units

        def outproj_chain(ct_cc, ft, nd):
            # out[ft*128:(ft+1)*128, nd*512:(nd+1)*512] partial
            units = []
            pw = [None]
            fl = ft % 4  # f-tile within the cc chunk

            def mk(jj):
                def emit():
                    if jj == 0:
                        pw[0] = ppw.tile([P, 512], FP32, tag="pw", name="pw")
                    nc.tensor.matmul(
                        pw[0], lhsT=ct_cc[:, jj, ts(fl, P)],
                        rhs=wo_sb[:, jj, ts(nd, 512)],
                        start=(jj == 0), stop=(jj == J - 1))
                return emit
            for jj in range(J):
                units.append(mk(jj))

            def drain():
                # SBUF bounce (PSUM cannot DMA directly; gpsimd cannot read it)
                ob = obpool.tile([P, 512], FP32, tag="ob", name="ob")
                nc.vector.tensor_copy(out=ob, in_=pw[0])
                nc.sync.dma_start(out=out[ts(ft, P), ds(nd * 512, 512)], in_=ob)
            units.append(drain)
            return units

        # ---------- prelude: V proj (+ K, Q(j0) eager) --------------------
        with tc.tile_pool(name="xvpool", bufs=1) as xvpool:
            xv_sb = xvpool.tile([P, DC, S], BF16, tag="xv", name="xv")
            xv_r = xv[:, :].rearrange("(a p) s -> p a s", p=P)
            wv_r = wv[:, :].rearrange("(a p) n -> p a n", p=P)
            for dc in range(DC):
                nc.sync.dma_start(out=wv_sb[:, dc, :], in_=wv_r[:, dc, :])
                nc.sync.dma_start(out=xv_sb[:, dc, 0:1024],
                                  in_=xv_r[:, dc, 0:1024])
            for dc in range(DC):
                nc.sync.dma_start(out=xv_sb[:, dc, 1024:2048],
                                  in_=xv_r[:, dc, 1024:2048])
            load_w(wk_sb, wk)
            load_x(xk_sb, xk)
            load_w(wq_sb, wq)
            load_x(xq_sb, xq)
            load_w(wo_sb, wo, "(j p) d -> p j d")
            for m in range(TT):
                ps = ppw.tile([P, HK], FP32, tag="pw", name="pw")
                for dc in range(DC):
                    nc.tensor.matmul(ps, lhsT=xv_sb[:, dc, ts(m, P)],
                                     rhs=wv_sb[:, dc, :],
                                     start=(dc == 0), stop=(dc == DC - 1))
                nc.vector.tensor_copy(
                    out=vt[m][:, :, 0:Dh],
                    in_=ps.rearrange("p (h k) -> p h k", h=H_LOC))
                nc.vector.memset(vt[m][:, :, Dh:Dh + 1], 1.0)

            # K proj j0 (full S) + Q proj (j0, cc0) eager
            for cc in range(NCC):
                for u in proj_chain(xk_sb, wk_sb, 0, cc, kT[0]):
                    u()
            for u in proj_chain(xq_sb, wq_sb, 0, 0, qT[0]):
                u()

        # weave supply: K(j)/Q(j, cc0) for j>0, then Q(*, cc) later;
        # outproj(cc) units are appended as each cc completes.
        for jj in range(1, J):
            weave_q.extend(proj_chain(xq_sb, wq_sb, jj, 0, qT[jj]))
            for cc in range(NCC):
                weave_q.extend(proj_chain(xk_sb, wk_sb, jj, cc, kT[jj]))

        # ---------- attention: cc outer, j inner --------------------------
        ct_by_cc = {}
        pending_epi = []

        def logits_pair(j, cc, m, pl):
            for h in range(2):
                nc.tensor.matmul(
                    pl[:, h, :],
                    lhsT=kT[j][64 * h:64 * h + 64, ts(m, P)],
                    rhs=qT[j][64 * h:64 * h + 64, ds(cc * W, W)],
                    start=True, stop=True)

        for cc in range(NCC):
            # queue next chunk's Q projections FIRST (they gate block
            # (cc+1, j) logits — must be fully emitted before then), then
            # the previous chunk's output projection.
            if cc + 1 < NCC:
                for jj in range(J):
                    weave_q.extend(
                        proj_chain(xq_sb, wq_sb, jj, cc + 1, qT[jj]))
            if cc >= 1:
                prev_ct = ct_by_cc[cc - 1]
                for ft in range((cc - 1) * 4, (cc - 1) * 4 + 4):
                    for nd in range(ND):
                        weave_q.extend(outproj_chain(prev_ct, ft, nd))
            ct_cc = ctpool.tile([P, J, W], BF16, tag="ct", name="ct_cc")
            ct_by_cc[cc] = ct_cc
            for j in range(J):
                rate = 5 if cc == 0 else 1
                pcA = ppc.tile([Dh + 1, W], FP32, tag="pcA", name="pcA")
                pcB = ppc.tile([Dh + 1, W], FP32, tag="pcB", name="pcB")
                pl = ppl.tile([P, 2, W], FP32, tag="pl", name="pl")
                logits_pair(j, cc, 0, pl)
                es = {}
                for m in range(TT):
                    e = epool.tile([P, 2, W], BF16, tag="e", name="e")
                    es[m] = e
                    if m in DVE_M:
                        nc.vector.tensor_scalar(
                            e[:, :, :].bitcast(I16), pl[:, :, :],
                            SCHR_A, SCHR_B, MULT, ADD)
                    else:
                        nc.scalar.activation(out=e, in_=pl, func=EXP,
                                             bias=bias_sh[:, :], scale=SCALE)
                    if m + 1 < TT:
                        pl = ppl.tile([P, 2, W], FP32, tag="pl", name="pl")
                        logits_pair(j, cc, m + 1, pl)
                    if m == 6:
                        # previous block's deferred normalization: far from
                        # both this block's early exps and pc-bank reuse
                        for fn in pending_epi:
                            fn()
                        pending_epi.clear()
                    weave(rate)
                    # ctx trails exp by one m-step so a late exp never
                    # stalls the PE queue
                    if m >= 1:
                        for h, pc in ((0, pcA), (1, pcB)):
                            nc.tensor.matmul(
                                pc, lhsT=vt[m - 1][:, 2 * j + h, 0:Dh + 1],
                                rhs=es[m - 1][:, h, :],
                                start=(m - 1 == 0), stop=False)
                        del es[m - 1]
                for h, pc in ((0, pcA), (1, pcB)):
                    nc.tensor.matmul(
                        pc, lhsT=vt[TT - 1][:, 2 * j + h, 0:Dh + 1],
                        rhs=es[TT - 1][:, h, :], start=False, stop=True)

                # ---- epilogue: drain pc now (frees banks), normalize
                # later (deferred) so the DVE/gpsimd chains never block
                # the next block's exp instructions.
                stA = stpool.tile([Dh + 1, W], FP32, tag="stA", name="stA")
                nc.vector.tensor_copy(out=stA, in_=pcA)
                stB = stpool.tile([Dh + 1, W], FP32, tag="stB", name="stB")
                nc.vector.tensor_copy(out=stB, in_=pcB)

                def make_epi(stA, stB, ct_cc, j):
                    def epi():
                        d0A = rpool.tile([1, W], FP32, tag="d0A", name="d0A")
                        nc.sync.dma_start(out=d0A, in_=stA[Dh:Dh + 1, :])
                        d0B = rpool.tile([1, W], FP32, tag="d0B", name="d0B")
                        nc.sync.dma_start(out=d0B, in_=stB[Dh:Dh + 1, :])
                        rA = rpool.tile([1, W], FP32, tag="rA", name="rA")
                        nc.vector.reciprocal_approx_fast(out=rA, in_=d0A)
                        rB = rpool.tile([1, W], FP32, tag="rB", name="rB")
                        nc.vector.reciprocal_approx_fast(out=rB, in_=d0B)
                        rbA = rpool.tile([Dh, W], FP32, tag="rbA", name="rbA")
                        nc.gpsimd.partition_broadcast(rbA, rA, channels=Dh)
                        rbB = rpool.tile([Dh, W], FP32, tag="rbB", name="rbB")
                        nc.gpsimd.partition_broadcast(rbB, rB, channels=Dh)
                        nc.vector.tensor_mul(out=ct_cc[0:64, j, :],
                                             in0=stA[0:Dh, :], in1=rbA)
                        tmB = stpool.tile([Dh, W], BF16, tag="tmB", name="tmB")
                        nc.vector.tensor_mul(out=tmB, in0=stB[0:Dh, :],
                                             in1=rbB)
                        nc.sync.dma_start(out=ct_cc[64:128, j, :], in_=tmB)
                    return epi
                make_epi(stA, stB, ct_cc, j)()

        # tail: flush last epilogue, then output projection of last chunk
        for fn in pending_epi:
            fn()
        pending_epi.clear()
        for ft in range((NCC - 1) * 4, (NCC - 1) * 4 + 4):
            for nd in range(ND):
                weave_q.extend(outproj_chain(ct_by_cc[NCC - 1], ft, nd))
        drain_all()

    nc.compile()
    return nc


def shard_inputs(query_input, key_input, value_input, Wq, Wk, Wv, Wo):
    """Per-core input maps: core c -> batch c//2, head group c%2."""
    in_maps = []
    for c in range(N_CORES):
        b, g = c // 2, c % 2
        hs = slice(g * H_LOC, (g + 1) * H_LOC)
        in_maps.append({
            "xq_t": np.ascontiguousarray(query_input[b].T).astype(BF),
            "xk_t": np.ascontiguousarray(key_input[b].T).astype(BF),
            "xv_t": np.ascontiguousarray(value_input[b].T).astype(BF),
            "wq": np.ascontiguousarray(Wq[:, hs, :]).reshape(D, HK).astype(BF),
            "wk": np.ascontiguousarray(Wk[:, hs, :]).reshape(D, HK).astype(BF),
            "wv": np.ascontiguousarray(Wv[:, hs, :]).reshape(D, HK).astype(BF),
            "wo": np.ascontiguousarray(Wo[hs]).reshape(HK, D).astype(BF),
        })
    return in_maps


_nc_cache = {}


def _get_nc():
    if "nc" not in _nc_cache:
        _nc_cache["nc"] = build_nc()
    return _nc_cache["nc"]


def run_spmd(inputs, trace=False, trace_cores=None):
    """Run the 8-core SPMD kernel; returns (output [B,S,D] fp32, results)."""
    from concourse.bass_utils import run_bass_kernel_spmd

    nc = _get_nc()
    in_maps = shard_inputs(**{k: np.asarray(v) for k, v in inputs.items()})
    res = run_bass_kernel_spmd(nc, in_maps, list(range(N_CORES)),
                               trace=trace, trace_cores=trace_cores)
    out = np.empty((B, S, D), np.float32)
    for b in range(B):
        out[b] = res.results[2 * b]["out_part"] + res.results[2 * b + 1]["out_part"]
    return out, res


def kernel(**inputs):
    out, _ = run_spmd(inputs)
    return out


# revision 26
# speedup vs baseline: 1.0626x; 1.0626x over previous
"""Multi-head attention (B=4, S=2048, D=1024, H=16, Dh=64) on 8 TRN2 NeuronCores.

Sharding: core c handles batch b = c // 2 and head group g = c % 2 (8 heads
each).  Every core computes Q/K/V projections for its batch+heads, the
attention for those heads, and a *partial* output projection (its heads'
slice of Wo).  The host sums the two partials per batch while unsharding.

v2 dataflow, engineered around three facts measured on hardware:
  - PE cost is (output free-size x cycle) regardless of M/K, and the PE
    only reaches 2.4 GHz after ~3us of *continuous* execution; any stall
    drops it to 1.2 GHz.  So the PE queue must never go idle.
  - Scalar ACT exp of all logits is a ~342us floor; part of the exp work
    is moved to the DVE as a Schraudolph bit-trick exp (affine -> int16
    convert -> bitcast bf16), calibrated to ~1.8% rms which cancels in
    softmax normalization (same approximated weights in numerator via the
    V ones-column denominator).
  - PSUM is 8 banks: logits tiles [128, 2 heads, 512f] double-buffered
    (4 banks) + two ctx accumulators [65, 512] (2) + weave psum (2).

Loop structure: f-chunk (cc) outer, head-pair (j) inner.  Background
matmul work (K/Q projections of later blocks, output projection of the
previous f-chunk) is "woven" between attention matmuls to fill the
PE bubbles left by exp latency, keeping the PE p-state at max.
"""

import sys

sys.path.insert(0, "/opt/trn_rl_repo")

import numpy as np
import ml_dtypes

BF = ml_dtypes.bfloat16

# Problem geometry (hardcoded; the harness always calls with these shapes).
B, S, D, H, Dh = 4, 2048, 1024, 16, 64
N_CORES = 8
H_LOC = H // 2          # heads per core
HK = H_LOC * Dh         # 512

P = 128
J = H_LOC // 2          # head pairs
DC = D // P             # contraction chunks for projections
TT = S // P             # t (key) tiles
W = 512                 # f-chunk width per head
NCC = S // W            # f-chunks
ND = D // 512

SCALE = float(Dh) ** -0.5      # 0.125
SHIFT = -2.0                   # exp(x*SCALE + SHIFT): keeps e in bf16 sweet spot
A16 = 128.0 / np.log(2.0)      # Schraudolph bf16 exponent stuffing
C16 = 7.3                      # sawtooth centering (calibrated)
SCHR_A = SCALE * A16
SCHR_B = 127 * 128 - C16 + SHIFT * A16
# m-tiles whose exp runs on the DVE (Schraudolph); rest on Scalar (exact)
DVE_M = (3, 8, 13)


def build_nc():
    import concourse.mybir as mybir
    import concourse.tile as tile
    from concourse import bacc
    from concourse.bass import ds, ts
    from contextlib import ExitStack

    FP32 = mybir.dt.float32
    BF16 = mybir.dt.bfloat16
    I16 = mybir.dt.int16
    EXP = mybir.ActivationFunctionType.Exp
    COPY = mybir.ActivationFunctionType.Copy
    MULT = mybir.AluOpType.mult
    ADD = mybir.AluOpType.add

    nc = bacc.Bacc("TRN2")
    xq = nc.declare_dram_parameter("xq_t", [D, S], BF16, isOutput=False)
    xk = nc.declare_dram_parameter("xk_t", [D, S], BF16, isOutput=False)
    xv = nc.declare_dram_parameter("xv_t", [D, S], BF16, isOutput=False)
    wq = nc.declare_dram_parameter("wq", [D, HK], BF16, isOutput=False)
    wk = nc.declare_dram_parameter("wk", [D, HK], BF16, isOutput=False)
    wv = nc.declare_dram_parameter("wv", [D, HK], BF16, isOutput=False)
    wo = nc.declare_dram_parameter("wo", [HK, D], BF16, isOutput=False)
    out = nc.declare_dram_parameter("out_part", [S, D], FP32, isOutput=True)

    with tile.TileContext(nc) as tc, ExitStack() as ctx:
        singles = ctx.enter_context(tc.tile_pool(name="singles", bufs=1))

        # ---- persistent SBUF tensors -------------------------------------
        wq_sb = singles.tile([P, DC, HK], BF16, tag="wq", name="wq")
        wk_sb = singles.tile([P, DC, HK], BF16, tag="wk", name="wk")
        wo_sb = singles.tile([P, J, D], BF16, tag="wo", name="wo")
        xq_sb = singles.tile([P, DC, S], BF16, tag="xq", name="xq")
        xk_sb = singles.tile([P, DC, S], BF16, tag="xk", name="xk")
        qT = [singles.tile([P, S], BF16, tag=f"qT{j}", name=f"qT{j}")
              for j in range(J)]
        kT = [singles.tile([P, S], BF16, tag=f"kT{j}", name=f"kT{j}")
              for j in range(J)]
        vt = [singles.tile([P, H_LOC, Dh + 1], BF16, tag=f"vt{m}", name=f"vt{m}")
              for m in range(TT)]
        bias_sh = singles.tile([P, 1], FP32, tag="bias_sh", name="bias_sh")
        nc.gpsimd.memset(bias_sh, SHIFT)

        # ---- DMA prelude (xv via transient pool, freed after V proj) -----
        def load_w(dst, src, rg="(a p) n -> p a n"):
            nc.sync.dma_start(out=dst, in_=src[:, :].rearrange(rg, p=P))

        def load_x(dst, src):
            src_r = src[:, :].rearrange("(a p) s -> p a s", p=P)
            for dc in range(DC):
                nc.sync.dma_start(out=dst[:, dc, :], in_=src_r[:, dc, :])

        # ---- pools -------------------------------------------------------
        ppl = ctx.enter_context(tc.tile_pool(name="ppl", bufs=2, space="PSUM"))
        ppc = ctx.enter_context(tc.tile_pool(name="ppc", bufs=1, space="PSUM"))
        ppw = ctx.enter_context(tc.tile_pool(name="ppw", bufs=2, space="PSUM"))
        epool = ctx.enter_context(tc.tile_pool(name="epool", bufs=2))
        ctpool = ctx.enter_context(tc.tile_pool(name="ctp", bufs=2))
        stpool = ctx.enter_context(tc.tile_pool(name="stage", bufs=1))
        rpool = ctx.enter_context(tc.tile_pool(name="rbc", bufs=1))
        obpool = ctx.enter_context(tc.tile_pool(name="outb", bufs=1))

        # ---------- weave machinery --------------------------------------
        # Background PE work (projection / output-projection matmuls) fed
        # one instruction at a time into attention's exp-latency bubbles.
        weave_q = []

        def weave(n):
            for _ in range(min(n, len(weave_q))):
                weave_q.pop(0)()

        def drain_all():
            while weave_q:
                weave_q.pop(0)()

        def proj_chain(x_sb, w_sb, jj, cc, dst):
            # kT/qT[jj][:, cc*W:(cc+1)*W] = (W[:, jj] ).T @ xT chunk
            units = []
            pw = [None]

            def mk(dc):
                def emit():
                    if dc == 0:
                        pw[0] = ppw.tile([P, W], FP32, tag="pw", name="pw")
                    nc.tensor.matmul(
                        pw[0], lhsT=w_sb[:, dc, ts(jj, P)],
                        rhs=x_sb[:, dc, ds(cc * W, W)],
                        start=(dc == 0), stop=(dc == DC - 1))
                return emit
            for dc in range(DC):
                units.append(mk(dc))

            def drain():
                # DVE (gpsimd cannot read PSUM); Scalar stays exp-only
                nc.vector.tensor_copy(out=dst[:, ds(cc * W, W)], in_=pw[0])
            units.append(drain)
            return units

        def outproj_chain(ct_cc, ft, nd):
            # out[ft*128:(ft+1)*128, nd*512:(nd+1)*512] partial
            units = []
            pw = [None]
            fl = ft % 4  # f-tile within the cc chunk

            def mk(jj):
                def emit():
                    if jj == 0:
                        pw[0] = ppw.tile([P, 512], FP32, tag="pw", name="pw")
                    nc.tensor.matmul(
                        pw[0], lhsT=ct_cc[:, jj, ts(fl, P)],
                        rhs=wo_sb[:, jj, ts(nd, 512)],
                        start=(jj == 0), stop=(jj == J - 1))
                return emit
            for jj in range(J):
                units.append(mk(jj))

            def drain():
                # SBUF bounce (PSUM cannot DMA directly; gpsimd cannot read it)
                ob = obpool.tile([P, 512], FP32, tag="ob", name="ob")
                nc.vector.tensor_copy(out=ob, in_=pw[0])
                nc.sync.dma_start(out=out[ts(ft, P), ds(nd * 512, 512)], in_=ob)
            units.append(drain)
            return units

        # ---------- prelude: V proj (+ K, Q(j0) eager) --------------------
        with tc.tile_pool(name="xvpool", bufs=1) as xvpool:
            # one tile per dc chunk: tile-granularity DMA deps mean the
            # first V-proj matmul only waits for its own chunk's DMA
            xv_r = xv[:, :].rearrange("(a p) s -> p a s", p=P)
            wv_r = wv[:, :].rearrange("(a p) n -> p a n", p=P)
            wv_t = []
            xv_t = []
            for dc in range(DC):
                wvd = xvpool.tile([P, HK], BF16, tag=f"wv{dc}", name=f"wv{dc}")
                nc.sync.dma_start(out=wvd, in_=wv_r[:, dc, :])
                wv_t.append(wvd)
                xvd = xvpool.tile([P, S], BF16, tag=f"xv{dc}", name=f"xv{dc}")
                nc.sync.dma_start(out=xvd, in_=xv_r[:, dc, :])
                xv_t.append(xvd)
            load_w(wk_sb, wk)
            load_x(xk_sb, xk)
            load_w(wq_sb, wq)
            load_x(xq_sb, xq)
            load_w(wo_sb, wo, "(j p) d -> p j d")
            for m in range(TT):
                ps = ppw.tile([P, HK], FP32, tag="pw", name="pw")
                for dc in range(DC):
                    nc.tensor.matmul(ps, lhsT=xv_t[dc][:, ts(m, P)],
                                     rhs=wv_t[dc],
                                     start=(dc == 0), stop=(dc == DC - 1))
                nc.vector.tensor_copy(
                    out=vt[m][:, :, 0:Dh],
                    in_=ps.rearrange("p (h k) -> p h k", h=H_LOC))
                nc.vector.memset(vt[m][:, :, Dh:Dh + 1], 1.0)

            # K proj j0 (full S) + Q proj (j0, cc0) eager
            for cc in range(NCC):
                for u in proj_chain(xk_sb, wk_sb, 0, cc, kT[0]):
                    u()
            for u in proj_chain(xq_sb, wq_sb, 0, 0, qT[0]):
                u()

        # weave supply: K(j)/Q(j, cc0) for j>0, then Q(*, cc) later;
        # outproj(cc) units are appended as each cc completes.
        for jj in range(1, J):
            weave_q.extend(proj_chain(xq_sb, wq_sb, jj, 0, qT[jj]))
            for cc in range(NCC):
                weave_q.extend(proj_chain(xk_sb, wk_sb, jj, cc, kT[jj]))

        # ---------- attention: cc outer, j inner --------------------------
        ct_by_cc = {}
        pending_epi = []

        def logits_pair(j, cc, m, pl):
            for h in range(2):
                nc.tensor.matmul(
                    pl[:, h, :],
                    lhsT=kT[j][64 * h:64 * h + 64, ts(m, P)],
                    rhs=qT[j][64 * h:64 * h + 64, ds(cc * W, W)],
                    start=True, stop=True)

        for cc in range(NCC):
            # queue next chunk's Q projections FIRST (they gate block
            # (cc+1, j) logits — must be fully emitted before then), then
            # the previous chunk's output projection.
            if cc + 1 < NCC:
                for jj in range(J):
                    weave_q.extend(
                        proj_chain(xq_sb, wq_sb, jj, cc + 1, qT[jj]))
            if cc >= 1:
                prev_ct = ct_by_cc[cc - 1]
                for ft in range((cc - 1) * 4, (cc - 1) * 4 + 4):
                    for nd in range(ND):
                        weave_q.extend(outproj_chain(prev_ct, ft, nd))
            ct_cc = ctpool.tile([P, J, W], BF16, tag="ct", name="ct_cc")
            ct_by_cc[cc] = ct_cc
            for j in range(J):
                rate = 5 if cc == 0 else 1
                pcA = ppc.tile([Dh + 1, W], FP32, tag="pcA", name="pcA")
                pcB = ppc.tile([Dh + 1, W], FP32, tag="pcB", name="pcB")
                pl = ppl.tile([P, 2, W], FP32, tag="pl", name="pl")
                logits_pair(j, cc, 0, pl)
                es = {}
                for m in range(TT):
                    e = epool.tile([P, 2, W], BF16, tag="e", name="e")
                    es[m] = e
                    if m in DVE_M:
                        nc.vector.tensor_scalar(
                            e[:, :, :].bitcast(I16), pl[:, :, :],
                            SCHR_A, SCHR_B, MULT, ADD)
                    else:
                        nc.scalar.activation(out=e, in_=pl, func=EXP,
                                             bias=bias_sh[:, :], scale=SCALE)
                    if m + 1 < TT:
                        pl = ppl.tile([P, 2, W], FP32, tag="pl", name="pl")
                        logits_pair(j, cc, m + 1, pl)
                    if m == 6:
                        # previous block's deferred normalization: far from
                        # both this block's early exps and pc-bank reuse
                        for fn in pending_epi:
                            fn()
                        pending_epi.clear()
                    weave(rate)
                    # ctx trails exp by one m-step so a late exp never
                    # stalls the PE queue
                    if m >= 1:
                        for h, pc in ((0, pcA), (1, pcB)):
                            nc.tensor.matmul(
                                pc, lhsT=vt[m - 1][:, 2 * j + h, 0:Dh + 1],
                                rhs=es[m - 1][:, h, :],
                                start=(m - 1 == 0), stop=False)
                        del es[m - 1]
                for h, pc in ((0, pcA), (1, pcB)):
                    nc.tensor.matmul(
                        pc, lhsT=vt[TT - 1][:, 2 * j + h, 0:Dh + 1],
                        rhs=es[TT - 1][:, h, :], start=False, stop=True)

                # ---- epilogue: drain pc now (frees banks), normalize
                # later (deferred) so the DVE/gpsimd chains never block
                # the next block's exp instructions.
                stA = stpool.tile([Dh + 1, W], FP32, tag="stA", name="stA")
                nc.vector.tensor_copy(out=stA, in_=pcA)
                stB = stpool.tile([Dh + 1, W], FP32, tag="stB", name="stB")
                nc.vector.tensor_copy(out=stB, in_=pcB)

                def make_epi(stA, stB, ct_cc, j):
                    def epi():
                        d0A = rpool.tile([1, W], FP32, tag="d0A", name="d0A")
                        nc.sync.dma_start(out=d0A, in_=stA[Dh:Dh + 1, :])
                        d0B = rpool.tile([1, W], FP32, tag="d0B", name="d0B")
                        nc.sync.dma_start(out=d0B, in_=stB[Dh:Dh + 1, :])
                        rA = rpool.tile([1, W], FP32, tag="rA", name="rA")
                        nc.vector.reciprocal_approx_fast(out=rA, in_=d0A)
                        rB = rpool.tile([1, W], FP32, tag="rB", name="rB")
                        nc.vector.reciprocal_approx_fast(out=rB, in_=d0B)
                        rbA = rpool.tile([Dh, W], FP32, tag="rbA", name="rbA")
                        nc.gpsimd.partition_broadcast(rbA, rA, channels=Dh)
                        rbB = rpool.tile([Dh, W], FP32, tag="rbB", name="rbB")
                        nc.gpsimd.partition_broadcast(rbB, rB, channels=Dh)
                        nc.vector.tensor_mul(out=ct_cc[0:64, j, :],
                                             in0=stA[0:Dh, :], in1=rbA)
                        tmB = stpool.tile([Dh, W], BF16, tag="tmB", name="tmB")
                        nc.vector.tensor_mul(out=tmB, in0=stB[0:Dh, :],
                                             in1=rbB)
                        nc.sync.dma_start(out=ct_cc[64:128, j, :], in_=tmB)
                    return epi
                make_epi(stA, stB, ct_cc, j)()

        # tail: flush last epilogue, then output projection of last chunk
        for fn in pending_epi:
            fn()
        pending_epi.clear()
        for ft in range((NCC - 1) * 4, (NCC - 1) * 4 + 4):
            for nd in range(ND):
                weave_q.extend(outproj_chain(ct_by_cc[NCC - 1], ft, nd))
        drain_all()

    nc.compile()
    return nc


def shard_inputs(query_input, key_input, value_input, Wq, Wk, Wv, Wo):
    """Per-core input maps: core c -> batch c//2, head group c%2."""
    in_maps = []
    for c in range(N_CORES):
        b, g = c // 2, c % 2
        hs = slice(g * H_LOC, (g + 1) * H_LOC)
        in_maps.append({
            "xq_t": np.ascontiguousarray(query_input[b].T).astype(BF),
            "xk_t": np.ascontiguousarray(key_input[b].T).astype(BF),
            "xv_t": np.ascontiguousarray(value_input[b].T).astype(BF),
            "wq": np.ascontiguousarray(Wq[:, hs, :]).reshape(D, HK).astype(BF),
            "wk": np.ascontiguousarray(Wk[:, hs, :]).reshape(D, HK).astype(BF),
            "wv": np.ascontiguousarray(Wv[:, hs, :]).reshape(D, HK).astype(BF),
            "wo": np.ascontiguousarray(Wo[hs]).reshape(HK, D).astype(BF),
        })
    return in_maps


_nc_cache = {}


def _get_nc():
    if "nc" not in _nc_cache:
        _nc_cache["nc"] = build_nc()
    return _nc_cache["nc"]


def run_spmd(inputs, trace=False, trace_cores=None):
    """Run the 8-core SPMD kernel; returns (output [B,S,D] fp32, results)."""
    from concourse.bass_utils import run_bass_kernel_spmd

    nc = _get_nc()
    in_maps = shard_inputs(**{k: np.asarray(v) for k, v in inputs.items()})
    res = run_bass_kernel_spmd(nc, in_maps, list(range(N_CORES)),
                               trace=trace, trace_cores=trace_cores)
    out = np.empty((B, S, D), np.float32)
    for b in range(B):
        out[b] = res.results[2 * b]["out_part"] + res.results[2 * b + 1]["out_part"]
    return out, res


def kernel(**inputs):
    out, _ = run_spmd(inputs)
    return out
